# revision 6
# baseline (speedup 1.0000x reference)
"""Multi-head attention kernel for 8 TRN2 NeuronCores.

Problem: x[4,2048,1024] -> qkv proj (w_qkv[1024,3072]) -> 16-head attention
(dim_head=64, scale=1024**-0.5) -> out proj (w_out[1024,1024] + b_out).

Sharding: core c in 0..7 handles batch b=c//2, head-group g=c%2 (8 heads).
Each core computes a partial output y_partial = attn_out_g @ w_out[rows_g];
host sums the pair (the tensor-parallel all-reduce, done at unshard time).

Layout strategy (zero on-chip transposes):
  - host supplies xT = x[b].T                     [1024, 2048] fp16
  - qkT = (x @ w_qk).T computed directly:  lhsT=w chunk, rhs=xT  -> [c, i]
  - V   = x @ w_v computed normally:       lhsT=xT chunk, rhs=wv -> [i, c]
  - S^T = k_h @ q_h^T per head:            lhsT=kT slice, rhs=qT slice
          -> [keys, q] so softmax's key-sum is a matmul contraction
  - P   = exp(S^T * scale)  (no max subtraction: |S*scale| < ~1)
  - O^T|s = [v_h | 1]^T @ P : lhsT=v[128,65] (ones col), rhs=P -> [65, q]
          row 64 is the softmax denominator s
  - normalize: O^T * (1/s) broadcast along partitions (gpsimd)
  - y = sum_h (O_h^T).T @ w_out_h : lhsT=otn[64,128], rhs=wo -> [i, e]
All matmul inputs fp16, PSUM accumulation fp32, output fp32.
"""

import numpy as np

B, N, D = 4, 2048, 1024
HEADS, DH = 16, 64
HP = HEADS // 2          # heads per core
GDIM = HP * DH           # 512 columns per head-group
SCALE = float(D) ** -0.5
NCORES = 8

_CACHE = {}


def _build():
    from contextlib import ExitStack

    import concourse.bass as bass
    import concourse.tile as tile
    from concourse import bacc, mybir

    F16 = mybir.dt.float16
    F32 = mybir.dt.float32
    EXP = mybir.ActivationFunctionType.Exp

    nc = bacc.Bacc(None, target_bir_lowering=False)

    xT_d = nc.declare_dram_parameter("xT", [D, N], F16, isOutput=False)
    wqk_d = nc.declare_dram_parameter("wqk", [D, 2 * GDIM], F16, isOutput=False)
    wv_d = nc.declare_dram_parameter("wv", [D, GDIM], F16, isOutput=False)
    wo_d = nc.declare_dram_parameter("wo", [HP, DH, D], F16, isOutput=False)
    bias_d = nc.declare_dram_parameter("bias", [D], F32, isOutput=False)
    out_d = nc.declare_dram_parameter("out", [N, D], F32, isOutput=True)

    with tile.TileContext(nc) as tc, ExitStack() as ctx:
        persist = ctx.enter_context(tc.tile_pool(name="persist", bufs=1))
        ptp = ctx.enter_context(tc.tile_pool(name="ptp", bufs=6))
        tiny = ctx.enter_context(tc.tile_pool(name="tiny", bufs=4))
        ypool = ctx.enter_context(tc.tile_pool(name="ypool", bufs=4))
        mm = ctx.enter_context(tc.tile_pool(name="mm", bufs=4, space="PSUM"))
        acc = ctx.enter_context(tc.tile_pool(name="acc", bufs=1, space="PSUM"))

        # ---- persistent SBUF tiles -------------------------------------
        xT = [persist.tile([128, N], F16, name=f"xT{e}", tag=f"xT{e}") for e in range(8)]
        wqk = [persist.tile([128, 2 * GDIM], F16, name=f"wqk{e}", tag=f"wqk{e}") for e in range(8)]
        wv = [persist.tile([128, GDIM], F16, name=f"wv{e}", tag=f"wv{e}") for e in range(8)]
        wo = [persist.tile([DH, D], F16, name=f"wo{h}", tag=f"wo{h}") for h in range(HP)]
        bias = persist.tile([128, D], F32, tag="bias")
        qkT = [persist.tile([128, N], F16, name=f"qkT{c}", tag=f"qkT{c}") for c in range(8)]
        vt = [persist.tile([128, HP, DH + 1], F16, name=f"v{kc}", tag=f"v{kc}") for kc in range(16)]
        otn = [persist.tile([DH, N], F16, name=f"otn{h}", tag=f"otn{h}") for h in range(HP)]

        for e in range(8):
            nc.sync.dma_start(out=xT[e], in_=xT_d[e * 128:(e + 1) * 128, :])
            nc.sync.dma_start(out=wqk[e], in_=wqk_d[e * 128:(e + 1) * 128, :])
            nc.sync.dma_start(out=wv[e], in_=wv_d[e * 128:(e + 1) * 128, :])
        for h in range(HP):
            nc.sync.dma_start(out=wo[h], in_=wo_d[h])
        bias_ap = bias_d[:]
        nc.sync.dma_start(
            out=bias,
            in_=bass.AP(tensor=bias_ap.tensor, offset=bias_ap.offset,
                        ap=[[0, 128]] + list(bias_ap.ap)),
        )
        # ones column (index DH) of each head block of v
        for kc in range(16):
            nc.vector.memset(vt[kc][:, :, DH:DH + 1], 1.0)
        # ones row at partition 64 for the reciprocal-broadcast matmul
        ones64 = persist.tile([65, DH], F16, tag="ones64")
        nc.vector.memset(ones64[64:65, :], 1.0)

        # ---- phase 1: qkv projections ----------------------------------
        for c in range(8):          # qkT chunks: 0-3 = qT, 4-7 = kT
            for i4 in range(4):
                ps = mm.tile([128, 512], F32, name="mmt", tag="mmt")
                for e in range(8):
                    nc.tensor.matmul(
                        ps, lhsT=wqk[e][:, c * 128:(c + 1) * 128],
                        rhs=xT[e][:, i4 * 512:(i4 + 1) * 512],
                        start=(e == 0), stop=(e == 7))
                nc.vector.tensor_copy(qkT[c][:, i4 * 512:(i4 + 1) * 512], ps)
        for it in range(16):        # V
            ps = mm.tile([128, 512], F32, name="mmt", tag="mmt")
            for e in range(8):
                nc.tensor.matmul(
                    ps, lhsT=xT[e][:, it * 128:(it + 1) * 128], rhs=wv[e],
                    start=(e == 0), stop=(e == 7))
            for h in range(HP):
                nc.vector.tensor_copy(vt[it][:, h, 0:DH],
                                      ps[:, h * DH:(h + 1) * DH])

        # ---- phase 2: attention per head -------------------------------
        for h in range(HP):
            qo = (h % 2) * 64
            qch, kch = h // 2, 4 + h // 2
            ot = [acc.tile([65, 512], F32, name=f"ot{h}_{i}", tag=f"ot{i}") for i in range(4)]
            for kc in range(16):
                for qc in range(4):
                    st = mm.tile([128, 512], F32, name="mmt", tag="mmt")
                    nc.tensor.matmul(
                        st,
                        lhsT=qkT[kch][qo:qo + 64, kc * 128:(kc + 1) * 128],
                        rhs=qkT[qch][qo:qo + 64, qc * 512:(qc + 1) * 512],
                        start=True, stop=True)
                    pt = ptp.tile([128, 512], F16)
                    nc.scalar.activation(pt, st, EXP, scale=SCALE)
                    nc.tensor.matmul(
                        ot[qc], lhsT=vt[kc][:, h, :], rhs=pt,
                        start=(kc == 0), stop=(kc == 15),
                        skip_group_check=True)
            for qc in range(4):
                rc = tiny.tile([65, 512], F16, name="rc", tag="rc")
                with nc.allow_low_precision(reason="1/s fits f16"):
                    nc.vector.reciprocal(rc[64:65, :], ot[qc][64:65, :])
                bcp = mm.tile([64, 512], F32, name="mmt", tag="mmt")
                nc.tensor.matmul(bcp, lhsT=ones64[64:65, :],
                                 rhs=rc[64:65, :], start=True, stop=True)
                bc = tiny.tile([64, 512], F32, name="bc", tag="bc")
                nc.vector.tensor_copy(bc, bcp)
                nc.vector.tensor_mul(otn[h][:, qc * 512:(qc + 1) * 512],
                                     ot[qc][0:64, :], bc)

        # ---- phase 3: output projection --------------------------------
        for it in range(16):
            for ec in range(2):
                ps = mm.tile([128, 512], F32, name="mmt", tag="mmt")
                for h in range(HP):
                    nc.tensor.matmul(
                        ps, lhsT=otn[h][:, it * 128:(it + 1) * 128],
                        rhs=wo[h][:, ec * 512:(ec + 1) * 512],
                        start=(h == 0), stop=(h == 7))
                yt = ypool.tile([128, 512], F32)
                nc.vector.tensor_add(yt, ps, bias[:, ec * 512:(ec + 1) * 512])
                nc.sync.dma_start(
                    out=out_d[it * 128:(it + 1) * 128, ec * 512:(ec + 1) * 512],
                    in_=yt)

    nc.compile()
    return nc


def _in_maps(x, w_qkv, w_out, b_out):
    x = np.asarray(x, dtype=np.float32)
    w_qkv = np.asarray(w_qkv, dtype=np.float32)
    w_out = np.asarray(w_out, dtype=np.float32)
    b_out = np.asarray(b_out, dtype=np.float32)
    maps = []
    for c in range(NCORES):
        b, g = c // 2, c % 2
        qcols = w_qkv[:, g * GDIM:(g + 1) * GDIM]
        kcols = w_qkv[:, D + g * GDIM:D + (g + 1) * GDIM]
        vcols = w_qkv[:, 2 * D + g * GDIM:2 * D + (g + 1) * GDIM]
        maps.append({
            "xT": np.ascontiguousarray(x[b].T).astype(np.float16),
            "wqk": np.concatenate([qcols, kcols], axis=1).astype(np.float16),
            "wv": np.ascontiguousarray(vcols).astype(np.float16),
            "wo": np.ascontiguousarray(
                w_out[g * GDIM:(g + 1) * GDIM, :].reshape(HP, DH, D)
            ).astype(np.float16),
            "bias": (b_out if g == 0 else np.zeros_like(b_out)),
        })
    return maps


def kernel(x, w_qkv, w_out, b_out):
    from concourse.bass_utils import run_bass_kernel_spmd

    if "nc" not in _CACHE:
        _CACHE["nc"] = _build()
    nc = _CACHE["nc"]
    maps = _in_maps(x, w_qkv, w_out, b_out)
    res = run_bass_kernel_spmd(nc, maps, core_ids=list(range(NCORES)))
    outs = res.results
    y = np.empty((B, N, D), dtype=np.float32)
    for b in range(B):
        y[b] = outs[2 * b]["out"] + outs[2 * b + 1]["out"]
    return y


# revision 7
# speedup vs baseline: 1.4304x; 1.4304x over previous
"""Multi-head attention kernel for 8 TRN2 NeuronCores.

Problem: x[4,2048,1024] -> qkv proj (w_qkv[1024,3072]) -> 16-head attention
(dim_head=64, scale=1024**-0.5) -> out proj (w_out[1024,1024] + b_out).

Sharding: core c in 0..7 handles batch b=c//2, head-group g=c%2 (8 heads).
Each core computes a partial output y_partial = attn_out_g @ w_out[rows_g];
host sums the pair (the tensor-parallel all-reduce, done at unshard time).

Layout strategy (zero on-chip transposes):
  - host supplies xT = x[b].T                     [1024, 2048] fp16
  - qkT = (x @ w_qk).T computed directly:  lhsT=w chunk, rhs=xT  -> [c, i]
  - V   = x @ w_v computed normally:       lhsT=xT chunk, rhs=wv -> [i, c]
  - S^T = k_h @ q_h^T per head:            lhsT=kT slice, rhs=qT slice
          -> [keys, q] so softmax's key-sum is a matmul contraction
  - P   = exp(S^T * scale)  (no max subtraction: |S*scale| < ~1)
  - O^T|s = [v_h | 1]^T @ P : lhsT=v[128,65] (ones col), rhs=P -> [65, q]
          row 64 is the softmax denominator s
  - normalize: 1/s broadcast via ones-matmul, applied off critical path
  - y = sum_h (O_h^T).T @ w_out_h : lhsT=otn[64,128], rhs=wo -> [i, e]
All matmul inputs fp16, PSUM accumulation fp32, output fp32.
"""

import numpy as np

B, N, D = 4, 2048, 1024
HEADS, DH = 16, 64
HP = HEADS // 2          # heads per core
GDIM = HP * DH           # 512 columns per head-group
SCALE = float(D) ** -0.5
NCORES = 8

_CACHE = {}


def _build():
    from contextlib import ExitStack

    import concourse.bass as bass
    import concourse.tile as tile
    from concourse import bacc, mybir

    F16 = mybir.dt.float16
    F32 = mybir.dt.float32
    EXP = mybir.ActivationFunctionType.Exp

    nc = bacc.Bacc(None, target_bir_lowering=False)

    xT_d = nc.declare_dram_parameter("xT", [D, N], F16, isOutput=False)
    wqk_d = nc.declare_dram_parameter("wqk", [D, 2 * GDIM], F16, isOutput=False)
    wv_d = nc.declare_dram_parameter("wv", [D, GDIM], F16, isOutput=False)
    wo_d = nc.declare_dram_parameter("wo", [HP, DH, D], F16, isOutput=False)
    bias_d = nc.declare_dram_parameter("bias", [D], F32, isOutput=False)
    out_d = nc.declare_dram_parameter("out", [N, D], F32, isOutput=True)

    with tile.TileContext(nc) as tc, ExitStack() as ctx:
        persist = ctx.enter_context(tc.tile_pool(name="persist", bufs=1))
        ptp = ctx.enter_context(tc.tile_pool(name="ptp", bufs=4))
        rawp = ctx.enter_context(tc.tile_pool(name="rawp", bufs=6))
        tiny = ctx.enter_context(tc.tile_pool(name="tiny", bufs=4))
        ypool = ctx.enter_context(tc.tile_pool(name="ypool", bufs=2))
        # PSUM: stq tag = [128,1024] (2 banks) x 3 bufs + 2 ot banks = 8
        mm = ctx.enter_context(tc.tile_pool(name="mm", bufs=3, space="PSUM"))
        acc = ctx.enter_context(tc.tile_pool(name="acc", bufs=1, space="PSUM"))

        # ---- persistent SBUF tiles -------------------------------------
        xT = [persist.tile([128, N], F16, name=f"xT{e}", tag=f"xT{e}")
              for e in range(8)]
        wqk = [persist.tile([128, 2 * GDIM], F16, name=f"wqk{e}", tag=f"wqk{e}")
               for e in range(8)]
        wv = [persist.tile([128, GDIM], F16, name=f"wv{e}", tag=f"wv{e}")
              for e in range(8)]
        wo = [persist.tile([DH, D], F16, name=f"wo{h}", tag=f"wo{h}")
              for h in range(HP)]
        bias = persist.tile([128, D], F32, tag="bias")
        qkT = [persist.tile([128, N], F16, name=f"qkT{c}", tag=f"qkT{c}")
               for c in range(8)]
        vt = [persist.tile([128, HP, DH + 1], F16, name=f"v{kc}", tag=f"v{kc}")
              for kc in range(16)]
        otn = [persist.tile([DH, N], F16, name=f"otn{h}", tag=f"otn{h}")
               for h in range(HP)]

        for e in range(8):
            nc.sync.dma_start(out=xT[e], in_=xT_d[e * 128:(e + 1) * 128, :])
            nc.sync.dma_start(out=wqk[e], in_=wqk_d[e * 128:(e + 1) * 128, :])
            nc.sync.dma_start(out=wv[e], in_=wv_d[e * 128:(e + 1) * 128, :])
        for h in range(HP):
            nc.sync.dma_start(out=wo[h], in_=wo_d[h])
        bias_ap = bias_d[:]
        nc.sync.dma_start(
            out=bias,
            in_=bass.AP(tensor=bias_ap.tensor, offset=bias_ap.offset,
                        ap=[[0, 128]] + list(bias_ap.ap)),
        )
        # ones column (index DH) of each head block of v
        for kc in range(16):
            nc.vector.memset(vt[kc][:, :, DH:DH + 1], 1.0)
        # ones row at partition 64 for the reciprocal-broadcast matmul
        ones64 = persist.tile([65, DH], F16, tag="ones64")
        nc.vector.memset(ones64[64:65, :], 1.0)

        # ---- phase 1: qkv projections ----------------------------------
        # qkT chunks: c 0-3 = qT rows, 4-7 = kT rows. lhsT reused across the
        # two 512-wide halves of each [128,1024] psum tile.
        for c in range(8):
            for ih in range(2):             # i half: queries [ih*1024, +1024)
                ps = mm.tile([128, 1024], F32, name="stq", tag="stq")
                for e in range(8):
                    w_sl = wqk[e][:, c * 128:(c + 1) * 128]
                    nc.tensor.matmul(
                        ps[:, 0:512], lhsT=w_sl,
                        rhs=xT[e][:, ih * 1024:ih * 1024 + 512],
                        start=(e == 0), stop=(e == 7))
                    nc.tensor.matmul(
                        ps[:, 512:1024], lhsT=w_sl,
                        rhs=xT[e][:, ih * 1024 + 512:(ih + 1) * 1024],
                        start=(e == 0), stop=(e == 7))
                nc.vector.tensor_copy(
                    qkT[c][:, ih * 1024:(ih + 1) * 1024], ps)
        for ih in range(8):                 # V: two key-tiles per psum tile
            ps = mm.tile([128, 1024], F32, name="stq", tag="stq")
            for e in range(8):
                nc.tensor.matmul(
                    ps[:, 0:512], lhsT=xT[e][:, (2 * ih) * 128:(2 * ih + 1) * 128],
                    rhs=wv[e], start=(e == 0), stop=(e == 7))
                nc.tensor.matmul(
                    ps[:, 512:1024], lhsT=xT[e][:, (2 * ih + 1) * 128:(2 * ih + 2) * 128],
                    rhs=wv[e], start=(e == 0), stop=(e == 7))
            for j in range(2):
                nc.vector.tensor_copy(
                    vt[2 * ih + j][:, :, 0:DH],
                    ps[:, j * 512:(j + 1) * 512].rearrange(
                        "p (h d) -> p h d", h=HP))

        # ---- phase 2: attention, per (head, q-half) --------------------
        for h in range(HP):
            qo = (h % 2) * 64
            qch, kch = h // 2, 4 + h // 2
            for qh in range(2):             # queries [qh*1024, +1024)
                ot = [acc.tile([65, 512], F32, name=f"ot{h}_{qh}_{j}",
                               tag=f"ot{j}") for j in range(2)]
                for kc in range(16):
                    stq = mm.tile([128, 1024], F32, name="stq", tag="stq")
                    kt_sl = qkT[kch][qo:qo + 64, kc * 128:(kc + 1) * 128]
                    nc.tensor.matmul(
                        stq[:, 0:512], lhsT=kt_sl,
                        rhs=qkT[qch][qo:qo + 64, qh * 1024:qh * 1024 + 512],
                        start=True, stop=True)
                    nc.tensor.matmul(
                        stq[:, 512:1024], lhsT=kt_sl,
                        rhs=qkT[qch][qo:qo + 64, qh * 1024 + 512:(qh + 1) * 1024],
                        start=True, stop=True)
                    pt = ptp.tile([128, 1024], F16, name="pt", tag="pt")
                    nc.scalar.activation(pt, stq, EXP, scale=SCALE)
                    v_sl = vt[kc][:, h, :]
                    nc.tensor.matmul(
                        ot[0], lhsT=v_sl, rhs=pt[:, 0:512],
                        start=(kc == 0), stop=(kc == 15),
                        skip_group_check=True)
                    nc.tensor.matmul(
                        ot[1], lhsT=v_sl, rhs=pt[:, 512:1024],
                        start=(kc == 0), stop=(kc == 15),
                        skip_group_check=True)
                # normalize: fast psum->sbuf copy frees the ot banks;
                # recip/broadcast/mul trail off the critical path.
                for j in range(2):
                    qc = 2 * qh + j
                    raw = rawp.tile([65, 512], F16, name="raw", tag="raw")
                    nc.vector.tensor_copy(raw, ot[j])
                    rc = tiny.tile([65, 512], F16, name="rc", tag="rc")
                    with nc.allow_low_precision(reason="1/s fits f16"):
                        nc.vector.reciprocal(rc[64:65, :], raw[64:65, :])
                    bcp = acc.tile([64, 512], F32, name="bcp", tag=f"ot{j}")
                    nc.tensor.matmul(bcp, lhsT=ones64[64:65, :],
                                     rhs=rc[64:65, :], start=True, stop=True)
                    bc = tiny.tile([64, 512], F16, name="bc", tag="bc")
                    nc.vector.tensor_copy(bc, bcp)
                    nc.vector.tensor_mul(
                        otn[h][:, qc * 512:(qc + 1) * 512],
                        raw[0:64, :], bc)

        # ---- phase 3: output projection --------------------------------
        for it in range(16):
            ps = mm.tile([128, 1024], F32, name="stq", tag="stq")
            for h in range(HP):
                o_sl = otn[h][:, it * 128:(it + 1) * 128]
                nc.tensor.matmul(ps[:, 0:512], lhsT=o_sl,
                                 rhs=wo[h][:, 0:512],
                                 start=(h == 0), stop=(h == 7))
                nc.tensor.matmul(ps[:, 512:1024], lhsT=o_sl,
                                 rhs=wo[h][:, 512:1024],
                                 start=(h == 0), stop=(h == 7))
            yt = ypool.tile([128, 1024], F32, name="yt", tag="yt")
            nc.vector.tensor_add(yt, ps, bias)
            nc.sync.dma_start(out=out_d[it * 128:(it + 1) * 128, :], in_=yt)

    nc.compile()
    return nc


def _in_maps(x, w_qkv, w_out, b_out):
    x = np.asarray(x, dtype=np.float32)
    w_qkv = np.asarray(w_qkv, dtype=np.float32)
    w_out = np.asarray(w_out, dtype=np.float32)
    b_out = np.asarray(b_out, dtype=np.float32)
    maps = []
    for c in range(NCORES):
        b, g = c // 2, c % 2
        qcols = w_qkv[:, g * GDIM:(g + 1) * GDIM]
        kcols = w_qkv[:, D + g * GDIM:D + (g + 1) * GDIM]
        vcols = w_qkv[:, 2 * D + g * GDIM:2 * D + (g + 1) * GDIM]
        maps.append({
            "xT": np.ascontiguousarray(x[b].T).astype(np.float16),
            "wqk": np.concatenate([qcols, kcols], axis=1).astype(np.float16),
            "wv": np.ascontiguousarray(vcols).astype(np.float16),
            "wo": np.ascontiguousarray(
                w_out[g * GDIM:(g + 1) * GDIM, :].reshape(HP, DH, D)
            ).astype(np.float16),
            "bias": (b_out if g == 0 else np.zeros_like(b_out)),
        })
    return maps


def kernel(x, w_qkv, w_out, b_out):
    from concourse.bass_utils import run_bass_kernel_spmd

    if "nc" not in _CACHE:
        _CACHE["nc"] = _build()
    nc = _CACHE["nc"]
    maps = _in_maps(x, w_qkv, w_out, b_out)
    res = run_bass_kernel_spmd(nc, maps, core_ids=list(range(NCORES)))
    outs = res.results
    y = np.empty((B, N, D), dtype=np.float32)
    for b in range(B):
        y[b] = outs[2 * b]["out"] + outs[2 * b + 1]["out"]
    return y
